# revision 1
# baseline (speedup 1.0000x reference)
import os, sys
import numpy as np

sys.path.insert(0, '/opt/trn_rl_repo')
from contextlib import ExitStack
import concourse.bass as bass
import concourse.tile as tile
from concourse import bacc, mybir
from concourse import bass_utils

F32 = mybir.dt.float32
AF = mybir.ActivationFunctionType
ALU = mybir.AluOpType
AX = mybir.AxisListType

S, B, E, H = 2048, 64, 256, 256
KN = 256
OUT = 10
NC = 8
BL = B // 4            # 16 batch per GRU core
SL = S // NC           # 256 seq per core for CNN / stage2
TB = 64                # GRU steps per block
NBLK = S // TB         # 32 blocks
H3 = 3 * H             # 768

_cache = {}


def _build_launch1():
    nc = bacc.Bacc("TRN2", target_bir_lowering=False, debug=False)
    idxg = nc.dram_tensor("idxg", (64, S * BL), F32, kind="ExternalInput")
    idxc = nc.dram_tensor("idxc", (64, SL * B + 8), F32, kind="ExternalInput")
    gtab = nc.dram_tensor("gtab", (64, H3), F32, kind="ExternalInput")
    whhT = nc.dram_tensor("whhT", (H, H3), F32, kind="ExternalInput")
    bhhn = nc.dram_tensor("bhhn", (128, 2), F32, kind="ExternalInput")
    h0p = nc.dram_tensor("h0p", (128, 2 * BL), F32, kind="ExternalInput")
    lkup = nc.dram_tensor("lkup", (64, E), F32, kind="ExternalInput")
    convT = nc.dram_tensor("convT", (E, 12 * KN), F32, kind="ExternalInput")
    convb = nc.dram_tensor("convb", (128, 6), F32, kind="ExternalInput")
    lwT = nc.dram_tensor("lwT", (3 * KN, 2 * H), F32, kind="ExternalInput")
    lb = nc.dram_tensor("lb", (128, 2 * H), F32, kind="ExternalInput")
    outT = nc.dram_tensor("outT", (H, S * BL), F32, kind="ExternalOutput")
    wproj = nc.dram_tensor("wproj", (SL, 2 * H), F32, kind="ExternalOutput")

    PAIRS = [(ki, k, j) for ki, k in enumerate((3, 4, 5)) for j in range(k)]

    with tile.TileContext(nc) as tc, ExitStack() as ctx:
        consts = ctx.enter_context(tc.tile_pool(name="consts", bufs=1))
        gxsb = ctx.enter_context(tc.tile_pool(name="gxsb", bufs=2))
        hists = ctx.enter_context(tc.tile_pool(name="hists", bufs=2))
        chain = ctx.enter_context(tc.tile_pool(name="chain", bufs=3))
        small = ctx.enter_context(tc.tile_pool(name="small", bufs=3))
        cnnsb = ctx.enter_context(tc.tile_pool(name="cnnsb", bufs=2))
        ghps = ctx.enter_context(tc.tile_pool(name="ghps", bufs=2, space="PSUM"))
        gxps = ctx.enter_context(tc.tile_pool(name="gxps", bufs=2, space="PSUM"))
        cnps = ctx.enter_context(tc.tile_pool(name="cnps", bufs=2, space="PSUM"))
        emps = ctx.enter_context(tc.tile_pool(name="emps", bufs=1, space="PSUM"))

        # ---- constants ----
        gtab_t = consts.tile([64, H3], F32)
        nc.sync.dma_start(gtab_t[:], gtab.ap())
        whh_t = [consts.tile([128, H3], F32, tag=f"whh{k}", name=f"whh{k}") for k in range(2)]
        for kk in range(2):
            nc.sync.dma_start(whh_t[kk][:], whhT.ap()[kk * 128:(kk + 1) * 128, :])
        bhhn_t = consts.tile([128, 2], F32)
        nc.sync.dma_start(bhhn_t[:], bhhn.ap())
        h0_t = consts.tile([128, 2 * BL], F32)
        nc.sync.dma_start(h0_t[:], h0p.ap())
        lkup_t = consts.tile([64, E], F32)
        nc.sync.dma_start(lkup_t[:], lkup.ap())
        ck = [consts.tile([128, 12 * KN], F32, tag=f"ck{k}", name=f"ck{k}") for k in range(2)]
        for kk in range(2):
            nc.sync.dma_start(ck[kk][:], convT.ap()[kk * 128:(kk + 1) * 128, :])
        convb_t = consts.tile([128, 6], F32)
        nc.sync.dma_start(convb_t[:], convb.ap())
        lw_t = consts.tile([128, 6 * 512], F32)
        for ci in range(6):
            nc.sync.dma_start(lw_t[:, ci * 512:(ci + 1) * 512],
                              lwT.ap()[ci * 128:(ci + 1) * 128, :])
        lb_t = consts.tile([128, 2 * H], F32)
        nc.sync.dma_start(lb_t[:], lb.ap())
        f_t = consts.tile([128, 6 * SL], F32)
        iota_i = consts.tile([64, 1], mybir.dt.int32)
        nc.gpsimd.iota(iota_i[:], [[0, 1]], base=0, channel_multiplier=1)
        iota_f = consts.tile([64, 1], F32)
        nc.vector.tensor_copy(iota_f[:], iota_i[:])

        def cnn_block(nb):
            # one-hot for 8 s-steps (512 cols) + 8 pad cols
            ixt = small.tile([64, 520], F32, tag="cidx")
            nc.sync.dma_start(ixt[:], idxc.ap()[:, nb * 512: nb * 512 + 520])
            oh = small.tile([64, 520], F32, tag="coh")
            nc.vector.tensor_scalar(oh[:], ixt[:],
                                    iota_f[:, 0:1], None, ALU.is_equal)
            emb = [cnnsb.tile([128, 520], F32, tag=f"emb{k}", name=f"emb{k}") for k in range(2)]
            for m in range(2):
                ep = emps.tile([128, 520], F32)
                nc.tensor.matmul(ep[:, 0:512], lkup_t[:, m * 128:(m + 1) * 128],
                                 oh[:, 0:512], start=True, stop=True)
                nc.tensor.matmul(ep[:, 512:520], lkup_t[:, m * 128:(m + 1) * 128],
                                 oh[:, 512:520], start=True, stop=True)
                nc.vector.tensor_copy(emb[m][:], ep[:])
            for ki, k in enumerate((3, 4, 5)):
                for m in range(2):
                    ci = ki * 2 + m
                    yp = cnps.tile([128, 512], F32, tag="convps")
                    mms = [(j, kk) for j in range(k) for kk in range(2)]
                    for ii, (j, kk) in enumerate(mms):
                        p = PAIRS.index((ki, k, j))
                        nc.tensor.matmul(
                            yp[:], ck[kk][:, p * KN + m * 128: p * KN + m * 128 + 128],
                            emb[kk][:, j: j + 512],
                            start=(ii == 0), stop=(ii == len(mms) - 1))
                    yr = cnnsb.tile([128, 512], F32, tag="yr")
                    nc.scalar.activation(yr[:], yp[:], AF.Relu,
                                         bias=convb_t[:, ci:ci + 1])
                    y3 = yr[:].rearrange("p (s b) -> p s b", b=64)
                    L = 64 - k + 1
                    nc.vector.memset(y3[:, :, L:64], 0.0)
                    nc.vector.tensor_reduce(
                        f_t[:, ci * SL + nb * 8: ci * SL + (nb + 1) * 8],
                        y3, AX.X, ALU.max)

        def gru_block(blk, hprev):
            ixt = small.tile([64, TB * BL], F32, tag="gidx")
            nc.sync.dma_start(ixt[:], idxg.ap()[:, blk * TB * BL:(blk + 1) * TB * BL])
            oh = small.tile([64, TB * BL], F32, tag="goh")
            nc.vector.tensor_scalar(oh[:], ixt[:],
                                    iota_f[:, 0:1], None, ALU.is_equal)
            gxb = gxsb.tile([128, TB * 6 * BL], F32)
            gxb3 = gxb[:].rearrange("p (t g) -> p t g", g=6 * BL)
            for m in range(6):
                for nb2 in range(2):
                    gp = gxps.tile([128, 512], F32)
                    nc.tensor.matmul(gp[:], gtab_t[:, m * 128:(m + 1) * 128],
                                     oh[:, nb2 * 512:(nb2 + 1) * 512],
                                     start=True, stop=True)
                    nc.vector.tensor_copy(
                        gxb3[:, nb2 * 32:(nb2 + 1) * 32, m * BL:(m + 1) * BL],
                        gp[:].rearrange("p (t g) -> p t g", g=BL))
            hist = hists.tile([128, TB * 2 * BL], F32)
            for tl in range(TB):
                t96 = tl * 6 * BL
                gh = ghps.tile([128, 6 * BL], F32)
                for m in range(6):
                    for kk in range(2):
                        nc.tensor.matmul(
                            gh[:, m * BL:(m + 1) * BL],
                            whh_t[kk][:, m * 128:(m + 1) * 128],
                            hprev[:, kk * BL:(kk + 1) * BL],
                            start=(kk == 0), stop=(kk == 1))
                ghnb = chain.tile([128, 2 * BL], F32, tag="ghnb")
                for kkk in range(2):
                    nc.vector.tensor_scalar_add(
                        ghnb[:, kkk * BL:(kkk + 1) * BL],
                        gh[:, 4 * BL + kkk * BL: 4 * BL + (kkk + 1) * BL],
                        bhhn_t[:, kkk:kkk + 1])
                prz = chain.tile([128, 4 * BL], F32, tag="prz")
                nc.vector.tensor_add(prz[:], gh[:, 0:4 * BL],
                                     gxb[:, t96: t96 + 4 * BL])
                rz = chain.tile([128, 4 * BL], F32, tag="rz")
                nc.scalar.activation(rz[:], prz[:], AF.Sigmoid)
                rghn = chain.tile([128, 2 * BL], F32, tag="rghn")
                nc.vector.tensor_mul(rghn[:], rz[:, 0:2 * BL], ghnb[:])
                prn = chain.tile([128, 2 * BL], F32, tag="prn")
                nc.vector.tensor_add(prn[:], rghn[:],
                                     gxb[:, t96 + 4 * BL: t96 + 6 * BL])
                nt = chain.tile([128, 2 * BL], F32, tag="nt")
                nc.scalar.activation(nt[:], prn[:], AF.Tanh)
                hmn = chain.tile([128, 2 * BL], F32, tag="hmn")
                nc.vector.tensor_sub(hmn[:], hprev[:], nt[:])
                zh = chain.tile([128, 2 * BL], F32, tag="zh")
                nc.vector.tensor_mul(zh[:], rz[:, 2 * BL:4 * BL], hmn[:])
                nc.vector.tensor_add(hist[:, tl * 2 * BL:(tl + 1) * 2 * BL],
                                     nt[:], zh[:])
                hprev = hist[:, tl * 2 * BL:(tl + 1) * 2 * BL]
            hist4 = hist[:].rearrange("p (t c g) -> p t c g", c=2, g=BL)
            for kk in range(2):
                nc.sync.dma_start(
                    outT.ap()[kk * 128:(kk + 1) * 128,
                              blk * TB * BL:(blk + 1) * TB * BL]
                    .rearrange("p (t g) -> p t g", g=BL),
                    hist4[:, :, kk, :])
            return hprev

        hprev = h0_t[:, 0:2 * BL]
        for blk in range(NBLK):
            cnn_block(blk)
            hprev = gru_block(blk, hprev)

        # wproj = f @ lwT + lb
        for sm in range(2):
            wp = cnps.tile([128, 512], F32, tag="convps")
            for ci in range(6):
                nc.tensor.matmul(wp[:], f_t[:, ci * SL + sm * 128: ci * SL + sm * 128 + 128],
                                 lw_t[:, ci * 512:(ci + 1) * 512],
                                 start=(ci == 0), stop=(ci == 5))
            wsb = cnnsb.tile([128, 512], F32, tag="wpsb")
            nc.vector.tensor_add(wsb[:], wp[:], lb_t[:])
            nc.sync.dma_start(wproj.ap()[sm * 128:(sm + 1) * 128, :], wsb[:])

    nc.compile()
    return nc


def _build_launch2():
    nc = bacc.Bacc("TRN2", target_bir_lowering=False, debug=False)
    NROW = SL * B  # 16384 rows (b-major: b*SL + sl)
    owT = nc.dram_tensor("owT", (2 * H, NROW), F32, kind="ExternalInput")
    wrep = nc.dram_tensor("wrep", (NROW, 2 * H), F32, kind="ExternalInput")
    wword = nc.dram_tensor("wword", (2 * H, 2 * H), F32, kind="ExternalInput")
    bword = nc.dram_tensor("bword", (128, 2 * H), F32, kind="ExternalInput")
    fcT = nc.dram_tensor("fcT", (2 * H, B * OUT), F32, kind="ExternalInput")
    attn = nc.dram_tensor("attn", (128, NROW // 128), F32, kind="ExternalOutput")
    gT = nc.dram_tensor("gT", (OUT, NROW), F32, kind="ExternalOutput")

    with tile.TileContext(nc) as tc, ExitStack() as ctx:
        consts = ctx.enter_context(tc.tile_pool(name="consts", bufs=1))
        owp = ctx.enter_context(tc.tile_pool(name="owp", bufs=3))
        work = ctx.enter_context(tc.tile_pool(name="work", bufs=3))
        psp = ctx.enter_context(tc.tile_pool(name="psp", bufs=2, space="PSUM"))
        gps = ctx.enter_context(tc.tile_pool(name="gps", bufs=2, space="PSUM"))

        ww_t = [consts.tile([128, 512], F32, tag=f"ww{k}", name=f"ww{k}") for k in range(4)]
        for kk in range(4):
            nc.sync.dma_start(ww_t[kk][:], wword.ap()[kk * 128:(kk + 1) * 128, :])
        bw_t = consts.tile([128, 512], F32)
        nc.sync.dma_start(bw_t[:], bword.ap())
        fct_t = [consts.tile([128, B * OUT], F32, tag=f"fct{k}", name=f"fct{k}") for k in range(4)]
        for kk in range(4):
            nc.sync.dma_start(fct_t[kk][:], fcT.ap()[kk * 128:(kk + 1) * 128, :])
        attn_sb = consts.tile([128, NROW // 128], F32, tag="attnsb")
        gt_sb = consts.tile([OUT, NROW], F32, tag="gtsb")

        for b in range(B):
            owb = [owp.tile([128, SL], F32, tag=f"owb{k}", name=f"owb{k}") for k in range(4)]
            for kk in range(4):
                nc.sync.dma_start(owb[kk][:],
                                  owT.ap()[kk * 128:(kk + 1) * 128, b * SL:(b + 1) * SL])
            for half in range(2):
                sq = psp.tile([128, 512], F32)
                for kk in range(4):
                    nc.tensor.matmul(sq[:], owb[kk][:, half * 128:(half + 1) * 128],
                                     ww_t[kk][:], start=(kk == 0), stop=(kk == 3))
                sqb = work.tile([128, 512], F32, tag="sqb")
                nc.vector.tensor_add(sqb[:], sq[:], bw_t[:])
                sqt = work.tile([128, 512], F32, tag="sqt")
                nc.scalar.activation(sqt[:], sqb[:], AF.Tanh)
                wr = work.tile([128, 512], F32, tag="wr")
                nc.sync.dma_start(wr[:], wrep.ap()[b * SL + half * 128:
                                                   b * SL + (half + 1) * 128, :])
                pr = work.tile([128, 512], F32, tag="pr")
                nc.vector.tensor_mul(pr[:], sqt[:], wr[:])
                nc.vector.tensor_reduce(attn_sb[:, 2 * b + half: 2 * b + half + 1],
                                        pr[:], AX.X, ALU.add)
            gp = gps.tile([OUT, SL], F32)
            for kk in range(4):
                nc.tensor.matmul(gp[:], fct_t[kk][:, b * OUT:(b + 1) * OUT],
                                 owb[kk][:], start=(kk == 0), stop=(kk == 3))
            nc.vector.tensor_copy(gt_sb[:, b * SL:(b + 1) * SL], gp[:])

        nc.sync.dma_start(attn.ap(), attn_sb[:])
        nc.sync.dma_start(gT.ap(), gt_sb[:])

    nc.compile()
    return nc


def kernel(embed, state_word, lookup,
           W_ih_f, W_hh_f, b_ih_f, b_hh_f,
           W_ih_b, W_hh_b, b_ih_b, b_hh_b,
           W_word, b_word,
           conv_w3, conv_b3, conv_w4, conv_b4, conv_w5, conv_b5,
           cnn_lin_w, cnn_lin_b, fc_w, fc_b):
    f32 = np.float32
    embed = np.asarray(embed)
    state_word = np.asarray(state_word, f32)
    lookup = np.asarray(lookup, f32)
    trace = os.environ.get("KTRACE") == "1"

    if "l1" not in _cache:
        _cache["l1"] = _build_launch1()
    if "l2" not in _cache:
        _cache["l2"] = _build_launch2()

    # ---- launch 1 host prep ----
    convT = np.concatenate(
        [np.asarray(w, f32)[:, :, j].T
         for w, k in ((conv_w3, 3), (conv_w4, 4), (conv_w5, 5)) for j in range(k)],
        axis=1)  # (E, 12*KN)
    convb = np.zeros((128, 6), f32)
    for ki, cb in enumerate((conv_b3, conv_b4, conv_b5)):
        cb = np.asarray(cb, f32)
        convb[:, ki * 2] = cb[0:128]
        convb[:, ki * 2 + 1] = cb[128:256]
    lwT = np.ascontiguousarray(np.asarray(cnn_lin_w, f32).T)      # (768, 512)
    lb = np.ascontiguousarray(np.broadcast_to(np.asarray(cnn_lin_b, f32), (128, 2 * H)))

    in_maps1 = []
    for c in range(NC):
        d = c // 4
        j = c % 4
        if d == 0:
            W_ih, W_hh, b_ih, b_hh = W_ih_f, W_hh_f, b_ih_f, b_hh_f
            idx = embed
        else:
            W_ih, W_hh, b_ih, b_hh = W_ih_b, W_hh_b, b_ih_b, b_hh_b
            idx = embed[::-1]
        W_ih = np.asarray(W_ih, f32); W_hh = np.asarray(W_hh, f32)
        b_ih = np.asarray(b_ih, f32); b_hh = np.asarray(b_hh, f32)
        G = W_ih @ lookup.T + b_ih[:, None]         # (768, 64)
        G[0:2 * H] += b_hh[0:2 * H, None]
        gtab = np.ascontiguousarray(G.T)            # (64, 768)
        bhhn = np.stack([b_hh[2 * H:2 * H + 128], b_hh[2 * H + 128:]], axis=1)
        h0 = state_word[d, j * BL:(j + 1) * BL, :]  # (16, 256)
        h0T = h0.T                                   # (256, 16)
        h0p = np.concatenate([h0T[0:128], h0T[128:256]], axis=1)  # (128, 32)
        idxg = np.ascontiguousarray(idx[:, j * BL:(j + 1) * BL]).astype(f32)
        idxc = np.zeros(SL * B + 8, f32)
        idxc[:SL * B] = embed[c * SL:(c + 1) * SL].astype(f32).ravel()
        in_maps1.append({
            "idxg": np.ascontiguousarray(np.broadcast_to(idxg.reshape(1, S * BL), (64, S * BL))),
            "idxc": np.ascontiguousarray(np.broadcast_to(idxc.reshape(1, -1), (64, SL * B + 8))),
            "gtab": gtab, "whhT": np.ascontiguousarray(W_hh.T),
            "bhhn": np.ascontiguousarray(bhhn), "h0p": np.ascontiguousarray(h0p),
            "lkup": lookup, "convT": np.ascontiguousarray(convT),
            "convb": convb, "lwT": lwT, "lb": lb,
        })
    import time as _t
    _t0 = _t.time()
    r1 = bass_utils.run_bass_kernel_spmd(_cache["l1"], in_maps1,
                                         core_ids=list(range(NC)), trace=trace)
    kernel.wall = [_t.time() - _t0]
    kernel.exec_ns = [r1.exec_time_ns]

    # ---- reassemble ow ----
    owT_full = np.empty((2 * H, S, B), f32)
    for c in range(NC):
        d, j = c // 4, c % 4
        o = r1.results[c]["outT"].reshape(H, S, BL)
        if d == 0:
            owT_full[0:H, :, j * BL:(j + 1) * BL] = o
        else:
            owT_full[H:2 * H, :, j * BL:(j + 1) * BL] = o[:, ::-1, :]
    wproj_full = np.concatenate([r1.results[c]["wproj"] for c in range(NC)], axis=0)

    # ---- launch 2 host prep ----
    W_word = np.asarray(W_word, f32)
    bword = np.ascontiguousarray(np.broadcast_to(np.asarray(b_word, f32)[:, 0], (128, 2 * H)))
    fcT = np.ascontiguousarray(
        np.asarray(fc_w, f32).reshape(OUT, B, 2 * H).transpose(2, 1, 0)
        .reshape(2 * H, B * OUT))
    in_maps2 = []
    for c in range(NC):
        sl = owT_full[:, c * SL:(c + 1) * SL, :]               # (512, 256, 64)
        owT_c = np.ascontiguousarray(sl.transpose(0, 2, 1).reshape(2 * H, SL * B))
        wrep = np.ascontiguousarray(np.tile(wproj_full[c * SL:(c + 1) * SL], (B, 1)))
        in_maps2.append({"owT": owT_c, "wrep": wrep, "wword": W_word,
                         "bword": bword, "fcT": fcT})
    _t1 = _t.time()
    r2 = bass_utils.run_bass_kernel_spmd(_cache["l2"], in_maps2,
                                         core_ids=list(range(NC)), trace=trace)
    kernel.wall.append(_t.time() - _t1)
    kernel.exec_ns.append(r2.exec_time_ns)

    # ---- host: tiny softmax + combine ----
    attn = np.empty((S, B), f32)
    g = np.empty((S, B, OUT), f32)
    for c in range(NC):
        a = r2.results[c]["attn"].T.reshape(B, SL)      # rows b-major
        attn[c * SL:(c + 1) * SL, :] = a.T
        gt = r2.results[c]["gT"].reshape(OUT, B, SL)
        g[c * SL:(c + 1) * SL] = gt.transpose(2, 1, 0)
    a = attn - attn.max(axis=0, keepdims=True)
    ea = np.exp(a)
    an = ea / ea.sum(axis=0, keepdims=True)
    logits = np.einsum('sb,sbo->so', an, g) + np.asarray(fc_b, f32)
    z = logits - logits.max(axis=-1, keepdims=True)
    ez = np.exp(z)
    return (ez / ez.sum(axis=-1, keepdims=True)).astype(f32)



# revision 5
# speedup vs baseline: 2.4417x; 2.4417x over previous
import os, sys
import numpy as np

sys.path.insert(0, '/opt/trn_rl_repo')
from contextlib import ExitStack
import concourse.bass as bass
import concourse.tile as tile
from concourse import bacc, mybir
from concourse import bass_utils

F32 = mybir.dt.float32
F16 = mybir.dt.float16
AF = mybir.ActivationFunctionType
ALU = mybir.AluOpType
AX = mybir.AxisListType

S, B, E, H = 2048, 64, 256, 256
KN = 256
OUT = 10
NC = 8
BL = B // 4            # 16 batch per GRU core
SL = S // NC           # 256 seq per core for CNN / stage2
TB = 64                # GRU steps per block
NBLK = S // TB         # 32 blocks
H3 = 3 * H             # 768

_cache = {}


def _build_launch1():
    nc = bacc.Bacc("TRN2", target_bir_lowering=False, debug=False)
    idxg = nc.dram_tensor("idxg", (64, S * BL), F32, kind="ExternalInput")
    idxc = nc.dram_tensor("idxc", (64, SL * B), F32, kind="ExternalInput")
    gall = nc.dram_tensor("gall", (64, H3), F16, kind="ExternalInput")
    gtabn = nc.dram_tensor("gtabn", (64, H), F16, kind="ExternalInput")
    whhT = nc.dram_tensor("whhT", (H, H3), F16, kind="ExternalInput")
    h0p = nc.dram_tensor("h0p", (128, 2 * BL), F16, kind="ExternalInput")
    lkup = nc.dram_tensor("lkup", (64, E), F16, kind="ExternalInput")
    convT = nc.dram_tensor("convT", (E, 12 * KN), F16, kind="ExternalInput")
    convb = nc.dram_tensor("convb", (128, 6), F32, kind="ExternalInput")
    lwT = nc.dram_tensor("lwT", (3 * KN, 2 * H), F16, kind="ExternalInput")
    lb = nc.dram_tensor("lb", (128, 2 * H), F32, kind="ExternalInput")
    outT = nc.dram_tensor("outT", (H, S * BL), F16, kind="ExternalOutput")
    wproj = nc.dram_tensor("wproj", (SL, 2 * H), F32, kind="ExternalOutput")

    PAIRS = [(ki, k, j) for ki, k in enumerate((3, 4, 5)) for j in range(k)]

    with tile.TileContext(nc) as tc, ExitStack() as ctx:
        consts = ctx.enter_context(tc.tile_pool(name="consts", bufs=1))
        blockp = ctx.enter_context(tc.tile_pool(name="blockp", bufs=2))
        hists = ctx.enter_context(tc.tile_pool(name="hists", bufs=2))
        chain = ctx.enter_context(tc.tile_pool(name="chain", bufs=3))
        cnnsb = ctx.enter_context(tc.tile_pool(name="cnnsb", bufs=2))
        gps = ctx.enter_context(tc.tile_pool(name="gps", bufs=2, space="PSUM"))
        cnps = ctx.enter_context(tc.tile_pool(name="cnps", bufs=2, space="PSUM"))
        shps = ctx.enter_context(tc.tile_pool(name="shps", bufs=2, space="PSUM"))

        # ---- constants ----
        gall_t = consts.tile([64, H3], F16)
        nc.sync.dma_start(gall_t[:], gall.ap())
        gtabn_t = consts.tile([64, H], F16)
        nc.sync.dma_start(gtabn_t[:], gtabn.ap())
        whh_t = [consts.tile([128, H3], F16, tag=f"whh{k}", name=f"whh{k}") for k in range(2)]
        for kk in range(2):
            nc.sync.dma_start(whh_t[kk][:], whhT.ap()[kk * 128:(kk + 1) * 128, :])
        h0_t = consts.tile([128, 2 * BL], F16)
        nc.sync.dma_start(h0_t[:], h0p.ap())
        lkup_t = consts.tile([64, E], F16)
        nc.sync.dma_start(lkup_t[:], lkup.ap())
        ck = [consts.tile([128, 12 * KN], F16, tag=f"ck{k}", name=f"ck{k}") for k in range(2)]
        for kk in range(2):
            nc.sync.dma_start(ck[kk][:], convT.ap()[kk * 128:(kk + 1) * 128, :])
        convb_t = consts.tile([128, 6], F32)
        nc.sync.dma_start(convb_t[:], convb.ap())
        lw_t = consts.tile([128, 6 * 512], F16)
        for ci in range(6):
            nc.sync.dma_start(lw_t[:, ci * 512:(ci + 1) * 512],
                              lwT.ap()[ci * 128:(ci + 1) * 128, :])
        lb_t = consts.tile([128, 2 * H], F32)
        nc.sync.dma_start(lb_t[:], lb.ap())
        f_t = consts.tile([128, 6 * SL], F16)
        iota_i = consts.tile([64, 1], mybir.dt.int32)
        nc.gpsimd.iota(iota_i[:], [[0, 1]], base=0, channel_multiplier=1)
        iota_f = consts.tile([64, 1], F32)
        nc.vector.tensor_copy(iota_f[:], iota_i[:])

        def cnn_block(nb):
            # one-hot for 8 s-steps (512 cols)
            ixt = blockp.tile([64, 512], F32, tag="cidx")
            nc.sync.dma_start(ixt[:], idxc.ap()[:, nb * 512:(nb + 1) * 512])
            oh = blockp.tile([64, 512], F16, tag="coh")
            nc.vector.tensor_scalar(oh[:], ixt[:],
                                    iota_f[:, 0:1], None, ALU.is_equal)
            emb = [cnnsb.tile([128, 512], F16, tag=f"emb{k}", name=f"emb{k}") for k in range(2)]
            for m in range(2):
                ep = shps.tile([128, 512], F32, tag="shp")
                nc.tensor.matmul(ep[:], lkup_t[:, m * 128:(m + 1) * 128],
                                 oh[:], start=True, stop=True)
                nc.vector.tensor_copy(emb[m][:], ep[:])
            for ki, k in enumerate((3, 4, 5)):
                for m in range(2):
                    ci = ki * 2 + m
                    yp = cnps.tile([128, 512], F32, tag="convps")
                    mms = [(j, kk) for j in range(k) for kk in range(2)]
                    for ii, (j, kk) in enumerate(mms):
                        p = PAIRS.index((ki, k, j))
                        nc.tensor.matmul(
                            yp[:, 0:512 - j],
                            ck[kk][:, p * KN + m * 128: p * KN + m * 128 + 128],
                            emb[kk][:, j:512],
                            start=(ii == 0), stop=(ii == len(mms) - 1))
                    yr = cnnsb.tile([128, 512], F16, tag="yr")
                    nc.scalar.activation(yr[:], yp[:], AF.Relu,
                                         bias=convb_t[:, ci:ci + 1])
                    y3 = yr[:].rearrange("p (s b) -> p s b", b=64)
                    L = 64 - k + 1
                    nc.vector.tensor_reduce(
                        f_t[:, ci * SL + nb * 8: ci * SL + (nb + 1) * 8],
                        y3[:, :, 0:L], AX.X, ALU.max)

        def gru_block(blk, hprev):
            # tokens + one-hot for 64 steps x 16 batch
            ixt = blockp.tile([64, TB * BL], F32, tag="gidx")
            nc.sync.dma_start(ixt[:], idxg.ap()[:, blk * TB * BL:(blk + 1) * TB * BL])
            ohg = blockp.tile([64, TB * BL], F16, tag="gohg")
            nc.vector.tensor_scalar(ohg[:], ixt[:],
                                    iota_f[:, 0:1], None, ALU.is_equal)
            # stage gx_n for the block: (128, TB*2*BL) fp16, cols (t, chunk, b)
            gxn = blockp.tile([128, TB * 2 * BL], F16, tag="gxn")
            gxn4 = gxn[:].rearrange("p (t c g) -> p t c g", c=2, g=BL)
            for hh in range(2):
                for ch in range(2):
                    gp = shps.tile([128, 512], F32, tag="shp")
                    nc.tensor.matmul(gp[:], gtabn_t[:, ch * 128:(ch + 1) * 128],
                                     ohg[:, hh * 512:(hh + 1) * 512],
                                     start=True, stop=True)
                    nc.vector.tensor_copy(
                        gxn4[:, hh * 32:(hh + 1) * 32, ch, :],
                        gp[:].rearrange("p (t g) -> p t g", g=BL))
            hist = hists.tile([128, TB * 2 * BL], F16)
            for tl in range(TB):
                oh_t = ohg[:, tl * BL:(tl + 1) * BL]
                ps_rz = gps.tile([128, 4 * BL], F32, tag="psrz")
                ps_n = gps.tile([128, 2 * BL], F32, tag="psn")
                # per gate-chunk accumulation group:
                #   W_hh[kk0] @ h  +  W_hh[kk1] @ h  +  gate-table @ onehot
                # (table carries gx_rz + b_ih + b_hh for r,z; b_hh_n for n)
                for m in range(4):
                    for kk in range(2):
                        nc.tensor.matmul(
                            ps_rz[:, m * BL:(m + 1) * BL],
                            whh_t[kk][:, m * 128:(m + 1) * 128],
                            hprev[:, kk * BL:(kk + 1) * BL],
                            start=(kk == 0), stop=False)
                    nc.tensor.matmul(ps_rz[:, m * BL:(m + 1) * BL],
                                     gall_t[:, m * 128:(m + 1) * 128], oh_t,
                                     start=False, stop=True)
                for m in range(2):
                    for kk in range(2):
                        nc.tensor.matmul(
                            ps_n[:, m * BL:(m + 1) * BL],
                            whh_t[kk][:, (4 + m) * 128:(5 + m) * 128],
                            hprev[:, kk * BL:(kk + 1) * BL],
                            start=(kk == 0), stop=False)
                    nc.tensor.matmul(ps_n[:, m * BL:(m + 1) * BL],
                                     gall_t[:, (4 + m) * 128:(5 + m) * 128], oh_t,
                                     start=False, stop=True)
                # rz = sigmoid(ps_rz); r = [:,0:2BL], z = [:,2BL:4BL]
                rz = chain.tile([128, 4 * BL], F16, tag="rz")
                nc.scalar.activation(rz[:], ps_rz[:], AF.Sigmoid)
                # off-path: a = z*h, omz = 1-z
                a_t = chain.tile([128, 2 * BL], F16, tag="at")
                nc.gpsimd.tensor_tensor(a_t[:], rz[:, 2 * BL:4 * BL], hprev[:],
                                        ALU.mult)
                omz = chain.tile([128, 2 * BL], F16, tag="omz")
                nc.gpsimd.tensor_scalar(omz[:], rz[:, 2 * BL:4 * BL],
                                        -1.0, 1.0, ALU.mult, ALU.add)
                # on-path
                rghn = chain.tile([128, 2 * BL], F16, tag="rghn")
                nc.vector.tensor_tensor(rghn[:], ps_n[:], rz[:, 0:2 * BL],
                                        ALU.mult)
                prn = chain.tile([128, 2 * BL], F16, tag="prn")
                nc.vector.tensor_add(prn[:], rghn[:],
                                     gxn[:, tl * 2 * BL:(tl + 1) * 2 * BL])
                nt = chain.tile([128, 2 * BL], F16, tag="nt")
                nc.scalar.activation(nt[:], prn[:], AF.Tanh)
                bt = chain.tile([128, 2 * BL], F16, tag="bt")
                nc.vector.tensor_tensor(bt[:], omz[:], nt[:], ALU.mult)
                nc.vector.tensor_add(hist[:, tl * 2 * BL:(tl + 1) * 2 * BL],
                                     bt[:], a_t[:])
                hprev = hist[:, tl * 2 * BL:(tl + 1) * 2 * BL]
            hist4 = hist[:].rearrange("p (t c g) -> p t c g", c=2, g=BL)
            for kk in range(2):
                nc.sync.dma_start(
                    outT.ap()[kk * 128:(kk + 1) * 128,
                              blk * TB * BL:(blk + 1) * TB * BL]
                    .rearrange("p (t g) -> p t g", g=BL),
                    hist4[:, :, kk, :])
            return hprev

        hprev = h0_t[:, 0:2 * BL]
        for blk in range(NBLK):
            cnn_block(blk)
            hprev = gru_block(blk, hprev)

        # wproj = f @ lwT + lb
        for sm in range(2):
            wp = cnps.tile([128, 512], F32, tag="convps")
            for ci in range(6):
                nc.tensor.matmul(wp[:], f_t[:, ci * SL + sm * 128: ci * SL + sm * 128 + 128],
                                 lw_t[:, ci * 512:(ci + 1) * 512],
                                 start=(ci == 0), stop=(ci == 5))
            wsb = cnnsb.tile([128, 512], F32, tag="wpsb")
            nc.vector.tensor_add(wsb[:], wp[:], lb_t[:])
            nc.sync.dma_start(wproj.ap()[sm * 128:(sm + 1) * 128, :], wsb[:])

    nc.compile()
    return nc


def _build_launch2():
    nc = bacc.Bacc("TRN2", target_bir_lowering=False, debug=False)
    NROW = SL * B  # 16384 rows (b-major: b*SL + sl)
    owT = nc.dram_tensor("owT", (2 * H, NROW), F16, kind="ExternalInput")
    wpro = nc.dram_tensor("wpro", (SL, 2 * H), F16, kind="ExternalInput")
    wword = nc.dram_tensor("wword", (2 * H, 2 * H), F16, kind="ExternalInput")
    bwrow = nc.dram_tensor("bwrow", (1, 2 * H), F16, kind="ExternalInput")
    fcT = nc.dram_tensor("fcT", (2 * H, B * OUT), F16, kind="ExternalInput")
    attn = nc.dram_tensor("attn", (128, NROW // 128), F32, kind="ExternalOutput")
    gT = nc.dram_tensor("gT", (OUT, NROW), F16, kind="ExternalOutput")

    with tile.TileContext(nc) as tc, ExitStack() as ctx:
        consts = ctx.enter_context(tc.tile_pool(name="consts", bufs=1))
        owp = ctx.enter_context(tc.tile_pool(name="owp", bufs=3))
        work = ctx.enter_context(tc.tile_pool(name="work", bufs=3))
        psp = ctx.enter_context(tc.tile_pool(name="psp", bufs=2, space="PSUM"))
        gps = ctx.enter_context(tc.tile_pool(name="gps", bufs=2, space="PSUM"))

        ww_t = [consts.tile([128, 512], F16, tag=f"ww{k}", name=f"ww{k}") for k in range(4)]
        for kk in range(4):
            nc.sync.dma_start(ww_t[kk][:], wword.ap()[kk * 128:(kk + 1) * 128, :])
        bw_t = consts.tile([1, 512], F16)
        nc.sync.dma_start(bw_t[:], bwrow.ap())
        ones1 = consts.tile([1, 128], F16)
        nc.vector.memset(ones1[:], 1.0)
        wpr_t = [consts.tile([128, 512], F16, tag=f"wpr{k}", name=f"wpr{k}") for k in range(2)]
        for kk in range(2):
            nc.sync.dma_start(wpr_t[kk][:], wpro.ap()[kk * 128:(kk + 1) * 128, :])
        fct_t = [consts.tile([128, B * OUT], F16, tag=f"fct{k}", name=f"fct{k}") for k in range(4)]
        for kk in range(4):
            nc.sync.dma_start(fct_t[kk][:], fcT.ap()[kk * 128:(kk + 1) * 128, :])
        attn_sb = consts.tile([128, NROW // 128], F32, tag="attnsb")
        gt_sb = consts.tile([OUT, NROW], F16, tag="gtsb")

        for b in range(B):
            owb = [owp.tile([128, SL], F16, tag=f"owb{k}", name=f"owb{k}") for k in range(4)]
            for kk in range(4):
                nc.sync.dma_start(owb[kk][:],
                                  owT.ap()[kk * 128:(kk + 1) * 128, b * SL:(b + 1) * SL])
            for half in range(2):
                sq = psp.tile([128, 512], F32)
                for kk in range(4):
                    nc.tensor.matmul(sq[:], owb[kk][:, half * 128:(half + 1) * 128],
                                     ww_t[kk][:], start=(kk == 0), stop=False)
                nc.tensor.matmul(sq[:], ones1[:], bw_t[:],
                                 start=False, stop=True)
                sqt = work.tile([128, 512], F16, tag="sqt")
                nc.scalar.activation(sqt[:], sq[:], AF.Tanh)
                pr = work.tile([128, 512], F16, tag="pr")
                nc.vector.scalar_tensor_tensor(
                    pr[:], sqt[:], 1.0, wpr_t[half][:],
                    ALU.mult, ALU.mult,
                    accum_out=attn_sb[:, 2 * b + half: 2 * b + half + 1])
            gp = gps.tile([OUT, SL], F32)
            for kk in range(4):
                nc.tensor.matmul(gp[:], fct_t[kk][:, b * OUT:(b + 1) * OUT],
                                 owb[kk][:], start=(kk == 0), stop=(kk == 3))
            nc.vector.tensor_copy(gt_sb[:, b * SL:(b + 1) * SL], gp[:])

        nc.sync.dma_start(attn.ap(), attn_sb[:])
        nc.sync.dma_start(gT.ap(), gt_sb[:])

    nc.compile()
    return nc


def kernel(embed, state_word, lookup,
           W_ih_f, W_hh_f, b_ih_f, b_hh_f,
           W_ih_b, W_hh_b, b_ih_b, b_hh_b,
           W_word, b_word,
           conv_w3, conv_b3, conv_w4, conv_b4, conv_w5, conv_b5,
           cnn_lin_w, cnn_lin_b, fc_w, fc_b):
    f32 = np.float32
    f16 = np.float16
    embed = np.asarray(embed)
    state_word = np.asarray(state_word, f32)
    lookup = np.asarray(lookup, f32)
    trace = os.environ.get("KTRACE") == "1"

    if "l1" not in _cache:
        _cache["l1"] = _build_launch1()
    if "l2" not in _cache:
        _cache["l2"] = _build_launch2()

    # ---- launch 1 host prep ----
    convT = np.concatenate(
        [np.asarray(w, f32)[:, :, j].T
         for w, k in ((conv_w3, 3), (conv_w4, 4), (conv_w5, 5)) for j in range(k)],
        axis=1)  # (E, 12*KN)
    convb = np.zeros((128, 6), f32)
    for ki, cb in enumerate((conv_b3, conv_b4, conv_b5)):
        cb = np.asarray(cb, f32)
        convb[:, ki * 2] = cb[0:128]
        convb[:, ki * 2 + 1] = cb[128:256]
    lwT = np.ascontiguousarray(np.asarray(cnn_lin_w, f32).T).astype(f16)  # (768, 512)
    lb = np.ascontiguousarray(np.broadcast_to(np.asarray(cnn_lin_b, f32), (128, 2 * H)))

    in_maps1 = []
    for c in range(NC):
        d = c // 4
        j = c % 4
        if d == 0:
            W_ih, W_hh, b_ih, b_hh = W_ih_f, W_hh_f, b_ih_f, b_hh_f
            idx = embed
        else:
            W_ih, W_hh, b_ih, b_hh = W_ih_b, W_hh_b, b_ih_b, b_hh_b
            idx = embed[::-1]
        W_ih = np.asarray(W_ih, f32); W_hh = np.asarray(W_hh, f32)
        b_ih = np.asarray(b_ih, f32); b_hh = np.asarray(b_hh, f32)
        G = W_ih @ lookup.T + b_ih[:, None]         # (768, 64)
        G[0:2 * H] += b_hh[0:2 * H, None]
        # gall: (64, 768) = [gx_rz + biases | b_hh_n broadcast]
        gall = np.concatenate(
            [G[0:2 * H].T, np.broadcast_to(b_hh[2 * H:], (64, H))], axis=1)
        gtabn = np.ascontiguousarray(G[2 * H:].T)   # (64, 256)
        h0 = state_word[d, j * BL:(j + 1) * BL, :]  # (16, 256)
        h0T = h0.T                                   # (256, 16)
        h0p = np.concatenate([h0T[0:128], h0T[128:256]], axis=1)  # (128, 32)
        idxg = np.ascontiguousarray(idx[:, j * BL:(j + 1) * BL]).astype(f32)
        idxc = embed[c * SL:(c + 1) * SL].astype(f32).ravel()
        in_maps1.append({
            "idxg": np.ascontiguousarray(np.broadcast_to(idxg.reshape(1, S * BL), (64, S * BL))),
            "idxc": np.ascontiguousarray(np.broadcast_to(idxc.reshape(1, -1), (64, SL * B))),
            "gall": np.ascontiguousarray(gall).astype(f16),
            "gtabn": gtabn.astype(f16),
            "whhT": np.ascontiguousarray(W_hh.T).astype(f16),
            "h0p": np.ascontiguousarray(h0p).astype(f16),
            "lkup": lookup.astype(f16),
            "convT": np.ascontiguousarray(convT).astype(f16),
            "convb": convb, "lwT": lwT, "lb": lb,
        })
    import time as _t
    _t0 = _t.time()
    r1 = bass_utils.run_bass_kernel_spmd(_cache["l1"], in_maps1,
                                         core_ids=list(range(NC)), trace=trace)
    kernel.wall = [_t.time() - _t0]
    kernel.exec_ns = [r1.exec_time_ns]

    # ---- reassemble ow ----
    owT_full = np.empty((2 * H, S, B), f16)
    for c in range(NC):
        d, j = c // 4, c % 4
        o = r1.results[c]["outT"].reshape(H, S, BL)
        if d == 0:
            owT_full[0:H, :, j * BL:(j + 1) * BL] = o
        else:
            owT_full[H:2 * H, :, j * BL:(j + 1) * BL] = o[:, ::-1, :]
    wproj_full = np.concatenate([r1.results[c]["wproj"] for c in range(NC)], axis=0)

    # ---- launch 2 host prep ----
    W_word = np.asarray(W_word, f32).astype(f16)
    bwrow = np.asarray(b_word, f32)[:, 0].reshape(1, 2 * H).astype(f16)
    fcT = np.ascontiguousarray(
        np.asarray(fc_w, f32).reshape(OUT, B, 2 * H).transpose(2, 1, 0)
        .reshape(2 * H, B * OUT)).astype(f16)
    in_maps2 = []
    for c in range(NC):
        sl = owT_full[:, c * SL:(c + 1) * SL, :]               # (512, 256, 64)
        owT_c = np.ascontiguousarray(sl.transpose(0, 2, 1).reshape(2 * H, SL * B))
        wpro = wproj_full[c * SL:(c + 1) * SL].astype(f16)
        in_maps2.append({"owT": owT_c, "wpro": np.ascontiguousarray(wpro),
                         "wword": W_word, "bwrow": bwrow, "fcT": fcT})
    _t1 = _t.time()
    r2 = bass_utils.run_bass_kernel_spmd(_cache["l2"], in_maps2,
                                         core_ids=list(range(NC)), trace=trace)
    kernel.wall.append(_t.time() - _t1)
    kernel.exec_ns.append(r2.exec_time_ns)

    # ---- host: tiny softmax + combine ----
    attn = np.empty((S, B), f32)
    g = np.empty((S, B, OUT), f32)
    for c in range(NC):
        a = r2.results[c]["attn"].T.reshape(B, SL)      # rows b-major
        attn[c * SL:(c + 1) * SL, :] = a.T
        gt = r2.results[c]["gT"].astype(f32).reshape(OUT, B, SL)
        g[c * SL:(c + 1) * SL] = gt.transpose(2, 1, 0)
    a = attn - attn.max(axis=0, keepdims=True)
    ea = np.exp(a)
    an = ea / ea.sum(axis=0, keepdims=True)
    logits = np.einsum('sb,sbo->so', an, g) + np.asarray(fc_b, f32)
    z = logits - logits.max(axis=-1, keepdims=True)
    ez = np.exp(z)
    return (ez / ez.sum(axis=-1, keepdims=True)).astype(f32)
